# revision 1
# baseline (speedup 1.0000x reference)
"""Multi-head self-attention (B=2, S=2048, D=1024, H=16, causal) on 8 TRN2 cores.

Sharding: core c handles batch b=c//4 and head-group g=c%4 (4 heads each).
Host pre-transposes x and the weight slices so the kernel never needs an
on-chip transpose:
  xT   [1024, 2048] = x[b].T
  wqT/wkT/wvT [1024, 256] = W.T[:, g*256:(g+1)*256]
  woT  [256, 1024] = Wo[:, g*256:(g+1)*256].T
Host sums the 4 per-group partial outputs per batch at the end.

On-chip dataflow per core (all matmul dtypes fp32r by default):
  qT/kT [256, 2048] (head dim on partitions), v [2048, 4*65] (with a ones
  column appended per head so the PV matmul also accumulates the softmax
  denominator in psum row 64).  Scores are computed transposed
  (scoresT[j, i]) so softmax needs no transpose at all; there is no
  max-subtraction (scores are O(+-6), exp is safe in fp32).
"""

import os
import sys

sys.path.insert(0, "/opt/trn_rl_repo")
os.environ.setdefault("MYCRO_LOCAL_CACHE", "1")

import numpy as np

import concourse.bacc as bacc
import concourse.bass as bass
import concourse.mybir as mybir
import concourse.tile as tile
from concourse import bass_utils

# The agent image's antenv lacks axon_hooks, so bass_utils' trace path dies on
# import.  Register a shim module that lazily builds the ctypes NTFF hook.
if "antenv.axon_hooks" not in sys.modules:
    import types

    _shim = types.ModuleType("antenv.axon_hooks")
    _shim._HOOK = None

    def _set_hook(hook, _m=_shim):
        _m._HOOK = hook

    def _get_hook(_m=_shim):
        if _m._HOOK is None:
            try:
                from trn_agent_boot.trn_boot import _ntff_profile_via_ctypes

                _m._HOOK = _ntff_profile_via_ctypes("/opt/axon/libaxon_pjrt.so")
            except Exception:
                _m._HOOK = None
        return _m._HOOK

    _shim.set_axon_ntff_profile_hook = _set_hook
    _shim.get_axon_ntff_profile_hook = _get_hook
    sys.modules["antenv.axon_hooks"] = _shim

B, S, D, H = 2, 2048, 1024, 16
DK = 64                      # head dim
HC = 4                       # heads per core
GC = HC * DK                 # 256 cols per head-group
N_CORES = 8
SCALE = 1.0 / np.sqrt(DK)    # 0.125

F32 = mybir.dt.float32
MM_DT = getattr(mybir.dt, os.environ.get("BASS_MM_DT", "float32r"))

TRACE = False
LAST_RESULTS = None


def _dram(ap):
    """Bitcast a DRAM fp32 AP to the matmul dtype for DMA into fp32r tiles."""
    if MM_DT == F32:
        return ap
    return ap.bitcast(MM_DT)


def build_bass():
    nc = bacc.Bacc("TRN2", target_bir_lowering=False, debug=False)

    xT_d = nc.dram_tensor("xT", [D, S], F32, kind="ExternalInput")
    wqT_d = nc.dram_tensor("wqT", [D, GC], F32, kind="ExternalInput")
    wkT_d = nc.dram_tensor("wkT", [D, GC], F32, kind="ExternalInput")
    wvT_d = nc.dram_tensor("wvT", [D, GC], F32, kind="ExternalInput")
    woT_d = nc.dram_tensor("woT", [GC, D], F32, kind="ExternalInput")
    mask_d = nc.dram_tensor("mask", [128, 4, 512], F32, kind="ExternalInput")
    out_d = nc.dram_tensor("out", [S, D], F32, kind="ExternalOutput")

    EXP = mybir.ActivationFunctionType.Exp

    with tile.TileContext(nc) as tc:
        with (
            nc.allow_low_precision(reason="fp32r tiles carry full fp32 storage"),
            tc.tile_pool(name="const", bufs=1) as const,
            tc.tile_pool(name="work", bufs=3) as work,
            tc.tile_pool(name="apool", bufs=2) as apool,
            tc.tile_pool(name="opool", bufs=2) as opool,
            tc.tile_pool(name="rpool", bufs=2) as rpool,
            tc.tile_pool(name="psmm", bufs=3, space="PSUM") as psmm,
            tc.tile_pool(name="psout", bufs=2, space="PSUM") as psout,
        ):
            # ---- load inputs -------------------------------------------------
            xT_dr = _dram(xT_d.rearrange("(o p) s -> p o s", p=128))
            xts = []
            for ko in range(8):
                xt = const.tile([128, S], MM_DT, name=f"xt{ko}")
                nc.sync.dma_start(xt[:], xT_dr[:, ko, :])
                xts.append(xt)
            wq = const.tile([128, 8, GC], MM_DT)
            nc.gpsimd.dma_start(wq[:], _dram(wqT_d.rearrange("(o p) m -> p o m", p=128)))
            # descriptor generation for the strided weight loads is slow; put
            # them on the gpsimd queue so they don't serialize behind xT/wq
            wk = const.tile([128, 8, GC], MM_DT)
            nc.gpsimd.dma_start(wk[:], _dram(wkT_d.rearrange("(o p) m -> p o m", p=128)))
            wv = const.tile([128, 8, GC], MM_DT)
            nc.gpsimd.dma_start(wv[:], _dram(wvT_d.rearrange("(o p) m -> p o m", p=128)))
            wo = const.tile([128, 2, D], MM_DT)
            nc.gpsimd.dma_start(wo[:], _dram(woT_d.rearrange("(o p) n -> p o n", p=128)))
            maskt = const.tile([128, 4, 512], F32)
            nc.gpsimd.dma_start(maskt[:], mask_d[:])

            ones_f = const.tile([128, 64], F32)
            nc.vector.memset(ones_f[:], 1.0)
            ones64 = const.tile([1, 64], MM_DT)
            nc.vector.tensor_copy(ones64[:], ones_f[0:1, :])

            # ---- projections -------------------------------------------------
            # qT/kT: per (head-pair mo, s-half sbh) tiles [128, 1024] so the
            # attention phase can start before all projections finish
            qts = [[const.tile([128, 1024], MM_DT, name=f"q{m}{s}")
                    for s in range(2)] for m in range(2)]
            kts = [[const.tile([128, 1024], MM_DT, name=f"k{m}{s}")
                    for s in range(2)] for m in range(2)]
            # v: per j-chunk tiles; per head: 64 value cols + 1 ones col
            vts = []
            for io in range(16):
                vt = const.tile([128, HC * 65], MM_DT, name=f"v{io}")
                nc.vector.tensor_copy(
                    vt.rearrange("p (h u) -> p h u", u=65)[:, :, 64],
                    ones_f[:, 0:4],
                )
                vts.append(vt)

            for w_sb, dst in ((wq, qts), (wk, kts)):
                for mo in range(2):
                    for sbh in range(2):
                        # one [128,2,512] psum; ko outer so the stationary
                        # weight is reused by the two sb matmuls (1 LDW / 2 MM)
                        ps = psmm.tile([128, 2, 512], F32, tag="mm")
                        for ko in range(8):
                            for sb2 in range(2):
                                sb = 2 * sbh + sb2
                                nc.tensor.matmul(
                                    ps[:, sb2, :],
                                    (w_sb[:, ko, mo * 128:(mo + 1) * 128]),
                                    (xts[ko][:, sb * 512:(sb + 1) * 512]),
                                    start=(ko == 0),
                                    stop=(ko == 7),
                                    skip_group_check=True,
                                )
                        nc.vector.tensor_copy(
                            dst[mo][sbh][:],
                            ps.rearrange("p a n -> p (a n)"),
                        )

            for io in range(16):
                ps = psmm.tile([128, 256], F32, tag="mm")
                for ko in range(8):
                    nc.tensor.matmul(
                        ps[:],
                        (xts[ko][:, io * 128:(io + 1) * 128]),
                        (wv[:, ko, :]),
                        start=(ko == 0),
                        stop=(ko == 7),
                    )
                nc.vector.tensor_copy(
                    vts[io].rearrange("p (h u) -> p h u", u=65)[:, :, 0:64],
                    ps.rearrange("p (h e) -> p h e", e=64),
                )

            # ---- attention + output projection, per 512-query block ---------
            for Q in range(4):
                i0 = Q * 512
                aT = apool.tile([128, 2, 512], MM_DT, tag="aT")
                for mo in range(2):
                    nchunks = (Q + 1) * 4
                    out_ps = [
                        psout.tile([65, 512], F32, tag="out", name=f"out_ps{_h}")
                        for _h in range(2)
                    ]
                    for jc in range(nchunks):
                        sc = psmm.tile([128, 2, 512], F32, tag="mm")
                        for hp in range(2):
                            nc.tensor.matmul(
                                sc[:, hp, :],
                                (kts[mo][jc // 8][hp * 64:(hp + 1) * 64,
                                       (jc % 8) * 128:(jc % 8 + 1) * 128]),
                                (qts[mo][Q // 2][hp * 64:(hp + 1) * 64,
                                       (Q % 2) * 512:(Q % 2 + 1) * 512]),
                                start=True,
                                stop=True,
                                skip_group_check=True,
                            )
                        ex = work.tile([128, 2, 512], MM_DT, tag="exp")
                        nc.scalar.activation(ex[:], sc[:], EXP, scale=SCALE)
                        if jc // 4 == Q:  # diagonal chunk: apply causal mask
                            o = jc - 4 * Q
                            for hp in range(2):
                                nc.vector.tensor_mul(
                                    ex[:, hp, :], ex[:, hp, :], maskt[:, o, :]
                                )
                        for hp in range(2):
                            h = 2 * mo + hp
                            nc.tensor.matmul(
                                out_ps[hp][:],
                                (vts[jc][:, h * 65:(h + 1) * 65]),
                                (ex[:, hp, :]),
                                start=(jc == 0),
                                stop=(jc == nchunks - 1),
                                skip_group_check=True,
                            )
                    for hp in range(2):
                        den = rpool.tile([1, 512], F32, tag="den")
                        nc.vector.tensor_copy(den[:], out_ps[hp][64:65, :])
                        rd_f = rpool.tile([1, 512], F32, tag="rdf")
                        nc.vector.reciprocal_approx_fast(out=rd_f[:], in_=den[:])
                        rd = rpool.tile([1, 512], MM_DT, tag="rd")
                        nc.vector.tensor_copy(rd[:], rd_f[:])
                        # broadcast 1/denom across 64 partitions via K=1 matmul
                        rdb = psmm.tile([64, 512], F32, tag="mm")
                        nc.tensor.matmul(
                            rdb[:], (ones64[:]), (rd[:]),
                            start=True, stop=True, skip_group_check=True,
                        )
                        # walrus only accepts fp32r-consumed TensorTensor when
                        # in0 is already fp32r: round both operands via copies
                        att = work.tile([64, 512], MM_DT, tag="att")
                        nc.vector.tensor_copy(att[:], out_ps[hp][0:64, :])
                        rdbs = work.tile([64, 512], MM_DT, tag="rdbs")
                        nc.vector.tensor_copy(rdbs[:], rdb[:])
                        nc.vector.tensor_mul(
                            aT[hp * 64:(hp + 1) * 64, mo, :],
                            att[:],
                            rdbs[:],
                        )

                # out-proj for this query block: partial[s, :] = a @ woT
                for so in range(4):
                    osb = opool.tile([128, D], F32, tag="osb")
                    po = psmm.tile([128, 2, 512], F32, tag="mm")
                    for co in range(2):
                        for nt in range(2):
                            nc.tensor.matmul(
                                po[:, nt, :],
                                (aT[:, co, so * 128:(so + 1) * 128]),
                                (wo[:, co, nt * 512:(nt + 1) * 512]),
                                start=(co == 0),
                                stop=(co == 1),
                                skip_group_check=True,
                            )
                    nc.vector.tensor_copy(
                        osb[:], po.rearrange("p a n -> p (a n)")
                    )
                    nc.sync.dma_start(
                        out_d.rearrange("(a p) n -> p a n", p=128)[:, Q * 4 + so, :],
                        osb[:],
                    )

    nc.compile()
    return nc


_NC = None


def _get_nc():
    global _NC
    if _NC is None:
        _NC = build_bass()
    return _NC


def _causal_mask():
    j = np.arange(128)[:, None, None]
    o = np.arange(4)[None, :, None]
    i = np.arange(512)[None, None, :]
    return ((o * 128 + j) <= i).astype(np.float32)


def kernel(in_features, Wq, Wk, Wv, Wo):
    global LAST_RESULTS
    nc = _get_nc()

    x = np.asarray(in_features, np.float32)
    Wq = np.asarray(Wq, np.float32)
    Wk = np.asarray(Wk, np.float32)
    Wv = np.asarray(Wv, np.float32)
    Wo = np.asarray(Wo, np.float32)
    mask = _causal_mask()

    in_maps = []
    for c in range(N_CORES):
        b, g = divmod(c, 4)
        cols = slice(g * GC, (g + 1) * GC)
        in_maps.append({
            "xT": np.ascontiguousarray(x[b].T),
            "wqT": np.ascontiguousarray(Wq.T[:, cols]),
            "wkT": np.ascontiguousarray(Wk.T[:, cols]),
            "wvT": np.ascontiguousarray(Wv.T[:, cols]),
            "woT": np.ascontiguousarray(Wo[:, cols].T),
            "mask": mask,
        })

    res = bass_utils.run_bass_kernel_spmd(
        nc, in_maps, core_ids=list(range(N_CORES)), trace=TRACE,
    )
    LAST_RESULTS = res
    parts = [res.results[c]["out"] for c in range(N_CORES)]
    out = np.stack([
        parts[4 * b] + parts[4 * b + 1] + parts[4 * b + 2] + parts[4 * b + 3]
        for b in range(B)
    ]).astype(np.float32)
    return out



# revision 4
# speedup vs baseline: 1.1873x; 1.1873x over previous
"""Multi-head self-attention (B=2, S=2048, D=1024, H=16, causal) on 8 TRN2 cores.

Sharding: core c handles batch b=c//4 and head-group g=c%4 (4 heads each).
Host pre-transposes x and the weight slices (and pre-casts them to bf16) so
the kernel never needs an on-chip transpose or cast:
  xT   [1024, 2048] = x[b].T                     (bf16)
  wqT/wkT/wvT [1024, 256] = W.T[:, g*256:(g+1)*256]  (bf16)
  woT  [256, 1024] = Wo[:, g*256:(g+1)*256].T    (bf16)
Host sums the 4 per-group partial outputs per batch at the end.

On-chip dataflow per core (all matmuls bf16 with fp32 PSUM accumulation):
  qT/kT [256, 2048] (head dim on partitions), v [2048, 4*65] (with a ones
  column appended per head so the PV matmul also accumulates the softmax
  denominator in psum row 64).  Scores are computed transposed
  (scoresT[j, i]) so softmax needs no transpose at all; there is no
  max-subtraction (scores are O(+-6), exp is safe in fp32).
"""

import os
import sys

sys.path.insert(0, "/opt/trn_rl_repo")
os.environ.setdefault("MYCRO_LOCAL_CACHE", "1")

import ml_dtypes
import numpy as np

import concourse.bacc as bacc
import concourse.bass as bass
import concourse.mybir as mybir
import concourse.tile as tile
from concourse import bass_utils

# The agent image's antenv lacks axon_hooks, so bass_utils' trace path dies on
# import.  Register a shim module that lazily builds the ctypes NTFF hook.
if "antenv.axon_hooks" not in sys.modules:
    import types

    _shim = types.ModuleType("antenv.axon_hooks")
    _shim._HOOK = None

    def _set_hook(hook, _m=_shim):
        _m._HOOK = hook

    def _get_hook(_m=_shim):
        if _m._HOOK is None:
            try:
                from trn_agent_boot.trn_boot import _ntff_profile_via_ctypes

                _m._HOOK = _ntff_profile_via_ctypes("/opt/axon/libaxon_pjrt.so")
            except Exception:
                _m._HOOK = None
        return _m._HOOK

    _shim.set_axon_ntff_profile_hook = _set_hook
    _shim.get_axon_ntff_profile_hook = _get_hook
    sys.modules["antenv.axon_hooks"] = _shim

B, S, D, H = 2, 2048, 1024, 16
DK = 64                      # head dim
HC = 4                       # heads per core
GC = HC * DK                 # 256 cols per head-group
N_CORES = 8
SCALE = 1.0 / np.sqrt(DK)    # 0.125

F32 = mybir.dt.float32
BF16 = mybir.dt.bfloat16

TRACE = False
LAST_RESULTS = None


def build_bass():
    nc = bacc.Bacc("TRN2", target_bir_lowering=False, debug=False)

    xT_d = nc.dram_tensor("xT", [D, S], BF16, kind="ExternalInput")
    wqT_d = nc.dram_tensor("wqT", [D, GC], BF16, kind="ExternalInput")
    wkT_d = nc.dram_tensor("wkT", [D, GC], BF16, kind="ExternalInput")
    wvT_d = nc.dram_tensor("wvT", [D, GC], BF16, kind="ExternalInput")
    woT_d = nc.dram_tensor("woT", [GC, D], BF16, kind="ExternalInput")
    mask_d = nc.dram_tensor("mask", [128, 4, 512], BF16, kind="ExternalInput")
    out_d = nc.dram_tensor("out", [S, D], F32, kind="ExternalOutput")

    EXP = mybir.ActivationFunctionType.Exp

    with tile.TileContext(nc) as tc:
        with (
            nc.allow_low_precision(reason="bf16 matmuls, fp32 accumulation"),
            tc.tile_pool(name="const", bufs=1) as const,
            tc.tile_pool(name="work", bufs=3) as work,
            tc.tile_pool(name="apool", bufs=2) as apool,
            tc.tile_pool(name="opool", bufs=2) as opool,
            tc.tile_pool(name="rpool", bufs=2) as rpool,
            tc.tile_pool(name="psmm", bufs=3, space="PSUM") as psmm,
            tc.tile_pool(name="psout", bufs=2, space="PSUM") as psout,
        ):
            # ---- load inputs -------------------------------------------------
            xT_dr = xT_d.rearrange("(o p) s -> p o s", p=128)
            xts = []
            for ko in range(8):
                xt = const.tile([128, S], BF16, name=f"xt{ko}")
                nc.sync.dma_start(xt[:], xT_dr[:, ko, :])
                xts.append(xt)
            wq = const.tile([128, 8, GC], BF16)
            nc.gpsimd.dma_start(wq[:], wqT_d.rearrange("(o p) m -> p o m", p=128))
            # descriptor generation for the strided weight loads is slow; put
            # them on the gpsimd queue so they don't serialize behind xT/wq
            wk = const.tile([128, 8, GC], BF16)
            nc.gpsimd.dma_start(wk[:], wkT_d.rearrange("(o p) m -> p o m", p=128))
            wv = const.tile([128, 8, GC], BF16)
            nc.gpsimd.dma_start(wv[:], wvT_d.rearrange("(o p) m -> p o m", p=128))
            wo = const.tile([128, 2, D], BF16)
            nc.gpsimd.dma_start(wo[:], woT_d.rearrange("(o p) n -> p o n", p=128))
            maskt = const.tile([128, 4, 512], BF16)
            nc.gpsimd.dma_start(maskt[:], mask_d[:])

            ones_f = const.tile([128, 64], F32)
            nc.vector.memset(ones_f[:], 1.0)
            ones64 = const.tile([1, 64], BF16)
            nc.vector.tensor_copy(ones64[:], ones_f[0:1, :])

            # ---- projections -------------------------------------------------
            # qT/kT: per (head-pair mo, s-half sbh) tiles [128, 1024] so the
            # attention phase can start before all projections finish
            qts = [[const.tile([128, 1024], BF16, name=f"q{m}{s}")
                    for s in range(2)] for m in range(2)]
            kts = [[const.tile([128, 1024], BF16, name=f"k{m}{s}")
                    for s in range(2)] for m in range(2)]
            # v: per j-chunk tiles; per head: 64 value cols + 1 ones col
            vts = []
            for io in range(16):
                vt = const.tile([128, HC * 65], BF16, name=f"v{io}")
                nc.vector.tensor_copy(
                    vt.rearrange("p (h u) -> p h u", u=65)[:, :, 64],
                    ones_f[:, 0:4],
                )
                vts.append(vt)

            for w_sb, dst in ((wq, qts), (wk, kts)):
                for mo in range(2):
                    for sbh in range(2):
                        # one [128,2,512] psum; ko outer so the stationary
                        # weight is reused by the two sb matmuls (1 LDW / 2 MM)
                        ps = psmm.tile([128, 2, 512], F32, tag="mm")
                        for ko in range(8):
                            for sb2 in range(2):
                                sb = 2 * sbh + sb2
                                nc.tensor.matmul(
                                    ps[:, sb2, :],
                                    (w_sb[:, ko, mo * 128:(mo + 1) * 128]),
                                    (xts[ko][:, sb * 512:(sb + 1) * 512]),
                                    start=(ko == 0),
                                    stop=(ko == 7),
                                    skip_group_check=True,
                                )
                        nc.vector.tensor_copy(
                            dst[mo][sbh][:],
                            ps.rearrange("p a n -> p (a n)"),
                        )

            for io in range(16):
                ps = psmm.tile([128, 256], F32, tag="mm")
                for ko in range(8):
                    nc.tensor.matmul(
                        ps[:],
                        (xts[ko][:, io * 128:(io + 1) * 128]),
                        (wv[:, ko, :]),
                        start=(ko == 0),
                        stop=(ko == 7),
                    )
                nc.scalar.copy(
                    vts[io].rearrange("p (h u) -> p h u", u=65)[:, :, 0:64],
                    ps.rearrange("p (h e) -> p h e", e=64),
                )

            # ---- attention + output projection, per 512-query block ---------
            for Q in range(4):
                i0 = Q * 512
                aT = apool.tile([128, 2, 512], BF16, tag="aT")
                for mo in range(2):
                    nchunks = (Q + 1) * 4
                    out_ps = [
                        psout.tile([65, 512], F32, tag="out", name=f"out_ps{_h}")
                        for _h in range(2)
                    ]
                    for jc in range(nchunks):
                        sc = psmm.tile([128, 2, 512], F32, tag="mm")
                        for hp in range(2):
                            nc.tensor.matmul(
                                sc[:, hp, :],
                                (kts[mo][jc // 8][hp * 64:(hp + 1) * 64,
                                       (jc % 8) * 128:(jc % 8 + 1) * 128]),
                                (qts[mo][Q // 2][hp * 64:(hp + 1) * 64,
                                       (Q % 2) * 512:(Q % 2 + 1) * 512]),
                                start=True,
                                stop=True,
                                skip_group_check=True,
                            )
                        ex = work.tile([128, 2, 512], BF16, tag="exp")
                        nc.scalar.activation(ex[:], sc[:], EXP, scale=SCALE)
                        if jc // 4 == Q:  # diagonal chunk: apply causal mask
                            o = jc - 4 * Q
                            for hp in range(2):
                                eng = nc.vector if hp == 0 else nc.gpsimd
                                eng.tensor_mul(
                                    ex[:, hp, :], ex[:, hp, :], maskt[:, o, :]
                                )
                        for hp in range(2):
                            h = 2 * mo + hp
                            nc.tensor.matmul(
                                out_ps[hp][:],
                                (vts[jc][:, h * 65:(h + 1) * 65]),
                                (ex[:, hp, :]),
                                start=(jc == 0),
                                stop=(jc == nchunks - 1),
                                skip_group_check=True,
                            )
                    for hp in range(2):
                        den = rpool.tile([1, 512], F32, tag="den")
                        nc.vector.tensor_copy(den[:], out_ps[hp][64:65, :])
                        rd_f = rpool.tile([1, 512], F32, tag="rdf")
                        nc.vector.reciprocal_approx_fast(out=rd_f[:], in_=den[:])
                        rd = rpool.tile([1, 512], BF16, tag="rd")
                        nc.vector.tensor_copy(rd[:], rd_f[:])
                        # broadcast 1/denom across 64 partitions via K=1 matmul
                        rdb = psmm.tile([64, 512], F32, tag="mm")
                        nc.tensor.matmul(
                            rdb[:], (ones64[:]), (rd[:]),
                            start=True, stop=True, skip_group_check=True,
                        )
                        att = work.tile([64, 512], BF16, tag="att")
                        nc.scalar.copy(att[:], out_ps[hp][0:64, :])
                        rdbs = work.tile([64, 512], BF16, tag="rdbs")
                        nc.vector.tensor_copy(rdbs[:], rdb[:])
                        nc.vector.tensor_mul(
                            aT[hp * 64:(hp + 1) * 64, mo, :],
                            att[:],
                            rdbs[:],
                        )

                # out-proj for this query block: partial[s, :] = a @ woT
                for so in range(4):
                    osb = opool.tile([128, D], F32, tag="osb")
                    po = psmm.tile([128, 2, 512], F32, tag="mm")
                    for co in range(2):
                        for nt in range(2):
                            nc.tensor.matmul(
                                po[:, nt, :],
                                (aT[:, co, so * 128:(so + 1) * 128]),
                                (wo[:, co, nt * 512:(nt + 1) * 512]),
                                start=(co == 0),
                                stop=(co == 1),
                                skip_group_check=True,
                            )
                    nc.vector.tensor_copy(
                        osb[:], po.rearrange("p a n -> p (a n)")
                    )
                    nc.sync.dma_start(
                        out_d.rearrange("(a p) n -> p a n", p=128)[:, Q * 4 + so, :],
                        osb[:],
                    )

    nc.compile()
    return nc


_NC = None


def _get_nc():
    global _NC
    if _NC is None:
        _NC = build_bass()
    return _NC


def _causal_mask():
    j = np.arange(128)[:, None, None]
    o = np.arange(4)[None, :, None]
    i = np.arange(512)[None, None, :]
    return ((o * 128 + j) <= i).astype(ml_dtypes.bfloat16)


def kernel(in_features, Wq, Wk, Wv, Wo):
    global LAST_RESULTS
    nc = _get_nc()

    bf = ml_dtypes.bfloat16
    x = np.asarray(in_features, np.float32)
    Wq = np.asarray(Wq, np.float32)
    Wk = np.asarray(Wk, np.float32)
    Wv = np.asarray(Wv, np.float32)
    Wo = np.asarray(Wo, np.float32)
    mask = _causal_mask()

    in_maps = []
    for c in range(N_CORES):
        b, g = divmod(c, 4)
        cols = slice(g * GC, (g + 1) * GC)
        in_maps.append({
            "xT": np.ascontiguousarray(x[b].T).astype(bf),
            "wqT": np.ascontiguousarray(Wq.T[:, cols]).astype(bf),
            "wkT": np.ascontiguousarray(Wk.T[:, cols]).astype(bf),
            "wvT": np.ascontiguousarray(Wv.T[:, cols]).astype(bf),
            "woT": np.ascontiguousarray(Wo[:, cols].T).astype(bf),
            "mask": mask,
        })

    res = bass_utils.run_bass_kernel_spmd(
        nc, in_maps, core_ids=list(range(N_CORES)), trace=TRACE,
    )
    LAST_RESULTS = res
    parts = [res.results[c]["out"] for c in range(N_CORES)]
    out = np.stack([
        parts[4 * b] + parts[4 * b + 1] + parts[4 * b + 2] + parts[4 * b + 3]
        for b in range(B)
    ]).astype(np.float32)
    return out


# revision 8
# speedup vs baseline: 1.2340x; 1.0393x over previous
"""Multi-head self-attention (B=2, S=2048, D=1024, H=16, causal) on 8 TRN2 cores.

Sharding: core c handles batch b=c//4 and head-group g=c%4 (4 heads each).
Host pre-transposes x and the weight slices (and pre-casts them to bf16) so
the kernel never needs an on-chip transpose or cast:
  xT   [1024, 2048] = x[b].T                     (bf16)
  wqT/wkT/wvT [1024, 256] = W.T[:, g*256:(g+1)*256]  (bf16)
  woT  [256, 1024] = Wo[:, g*256:(g+1)*256].T    (bf16)
Host sums the 4 per-group partial outputs per batch at the end.

On-chip dataflow per core (all matmuls bf16 with fp32 PSUM accumulation):
  qT/kT [256, 2048] (head dim on partitions), v [2048, 4*65] (with a ones
  column appended per head so the PV matmul also accumulates the softmax
  denominator in psum row 64).  Scores are computed transposed
  (scoresT[j, i]) so softmax needs no transpose at all; there is no
  max-subtraction (scores are O(+-6), exp is safe in fp32).

Causal structure per 512-query block Q: keys below the diagonal region
(chunks jc < 4Q) are computed full-width with no masking; the diagonal
512x512 region is processed per key-chunk kc with a shrinking query range
(queries kc*128..512), so only the 4 true diagonal 128x128 blocks need an
element mask and the strictly-above-diagonal blocks are never computed.
"""

import os
import sys

sys.path.insert(0, "/opt/trn_rl_repo")
os.environ.setdefault("MYCRO_LOCAL_CACHE", "1")

import ml_dtypes
import numpy as np

import concourse.bacc as bacc
import concourse.bass as bass
import concourse.mybir as mybir
import concourse.tile as tile
from concourse import bass_utils

# The agent image's antenv lacks axon_hooks, so bass_utils' trace path dies on
# import.  Register a shim module that lazily builds the ctypes NTFF hook.
if "antenv.axon_hooks" not in sys.modules:
    import types

    _shim = types.ModuleType("antenv.axon_hooks")
    _shim._HOOK = None

    def _set_hook(hook, _m=_shim):
        _m._HOOK = hook

    def _get_hook(_m=_shim):
        if _m._HOOK is None:
            try:
                from trn_agent_boot.trn_boot import _ntff_profile_via_ctypes

                _m._HOOK = _ntff_profile_via_ctypes("/opt/axon/libaxon_pjrt.so")
            except Exception:
                _m._HOOK = None
        return _m._HOOK

    _shim.set_axon_ntff_profile_hook = _set_hook
    _shim.get_axon_ntff_profile_hook = _get_hook
    sys.modules["antenv.axon_hooks"] = _shim

B, S, D, H = 2, 2048, 1024, 16
DK = 64                      # head dim
HC = 4                       # heads per core
GC = HC * DK                 # 256 cols per head-group
N_CORES = 8
SCALE = 1.0 / np.sqrt(DK)    # 0.125

F32 = mybir.dt.float32
BF16 = mybir.dt.bfloat16

TRACE = False
LAST_RESULTS = None


def build_bass():
    nc = bacc.Bacc("TRN2", target_bir_lowering=False, debug=False)

    xT_d = nc.dram_tensor("xT", [D, S], BF16, kind="ExternalInput")
    wqT_d = nc.dram_tensor("wqT", [D, GC], BF16, kind="ExternalInput")
    wkT_d = nc.dram_tensor("wkT", [D, GC], BF16, kind="ExternalInput")
    wvT_d = nc.dram_tensor("wvT", [D, GC], BF16, kind="ExternalInput")
    woT_d = nc.dram_tensor("woT", [GC, D], BF16, kind="ExternalInput")
    mask_d = nc.dram_tensor("mask", [128, 2, 128], BF16, kind="ExternalInput")
    out_d = nc.dram_tensor("out", [S, D], F32, kind="ExternalOutput")

    EXP = mybir.ActivationFunctionType.Exp

    with tile.TileContext(nc) as tc:
        with (
            nc.allow_low_precision(reason="bf16 matmuls, fp32 accumulation"),
            tc.tile_pool(name="const", bufs=1) as const,
            tc.tile_pool(name="work", bufs=3) as work,
            tc.tile_pool(name="apool", bufs=2) as apool,
            tc.tile_pool(name="opool", bufs=2) as opool,
            tc.tile_pool(name="rpool", bufs=2) as rpool,
            tc.tile_pool(name="psmm", bufs=3, space="PSUM") as psmm,
            tc.tile_pool(name="psout", bufs=2, space="PSUM") as psout,
        ):
            # ---- load inputs -------------------------------------------------
            # x striped over 4 DMA queues so the load phase is short
            xT_dr = xT_d.rearrange("(o p) s -> p o s", p=128)
            dma_engines = [nc.sync, nc.scalar]
            xts = []
            for ko in range(8):
                xt = const.tile([128, S], BF16, name=f"xt{ko}")
                dma_engines[ko % 2].dma_start(xt[:], xT_dr[:, ko, :])
                xts.append(xt)
            wq = const.tile([128, 8, GC], BF16)
            nc.gpsimd.dma_start(wq[:], wqT_d.rearrange("(o p) m -> p o m", p=128))
            # descriptor generation for the strided weight loads is slow; put
            # them on the gpsimd queue so they don't serialize behind xT/wq
            wk = const.tile([128, 8, GC], BF16)
            nc.gpsimd.dma_start(wk[:], wkT_d.rearrange("(o p) m -> p o m", p=128))
            wv = const.tile([128, 8, GC], BF16)
            nc.gpsimd.dma_start(wv[:], wvT_d.rearrange("(o p) m -> p o m", p=128))
            wo = const.tile([128, 2, D], BF16)
            nc.gpsimd.dma_start(wo[:], woT_d.rearrange("(o p) n -> p o n", p=128))
            maskt = const.tile([128, 2, 128], BF16)
            nc.gpsimd.dma_start(maskt[:], mask_d[:])

            ones_f = const.tile([128, 64], F32)
            nc.vector.memset(ones_f[:], 1.0)
            ones64 = const.tile([1, 64], BF16)
            nc.vector.tensor_copy(ones64[:], ones_f[0:1, :])

            # ---- projections -------------------------------------------------
            # qT/kT: per (head-pair mo, s-half sbh) tiles [128, 1024] so the
            # attention phase can start before all projections finish
            qts = [[const.tile([128, 1024], BF16, name=f"q{m}{s}")
                    for s in range(2)] for m in range(2)]
            kts = [[const.tile([128, 1024], BF16, name=f"k{m}{s}")
                    for s in range(2)] for m in range(2)]
            # v: per j-chunk tiles; per head: 64 value cols + 1 ones col
            vts = []
            for io in range(16):
                vt = const.tile([128, HC * 65], BF16, name=f"v{io}")
                nc.vector.tensor_copy(
                    vt.rearrange("p (h u) -> p h u", u=65)[:, :, 64],
                    ones_f[:, 0:4],
                )
                vts.append(vt)

            for w_sb, dst in ((wq, qts), (wk, kts)):
                for mo in range(2):
                    for sbh in range(2):
                        # one [128,2,512] psum; ko outer so the stationary
                        # weight is reused by the two sb matmuls (1 LDW / 2 MM)
                        ps = psmm.tile([128, 2, 512], F32, tag="mm")
                        for ko in range(8):
                            for sb2 in range(2):
                                sb = 2 * sbh + sb2
                                nc.tensor.matmul(
                                    ps[:, sb2, :],
                                    (w_sb[:, ko, mo * 128:(mo + 1) * 128]),
                                    (xts[ko][:, sb * 512:(sb + 1) * 512]),
                                    start=(ko == 0),
                                    stop=(ko == 7),
                                    skip_group_check=True,
                                )
                        nc.vector.tensor_copy(
                            dst[mo][sbh][:],
                            ps.rearrange("p a n -> p (a n)"),
                        )

            for io in range(16):
                ps = psmm.tile([128, 256], F32, tag="mm")
                for ko in range(8):
                    nc.tensor.matmul(
                        ps[:],
                        (xts[ko][:, io * 128:(io + 1) * 128]),
                        (wv[:, ko, :]),
                        start=(ko == 0),
                        stop=(ko == 7),
                    )
                nc.scalar.copy(
                    vts[io].rearrange("p (h u) -> p h u", u=65)[:, :, 0:64],
                    ps.rearrange("p (h e) -> p h e", e=64),
                )

            # ---- attention + output projection, per 512-query block ---------
            for Q in range(4):
                aT = apool.tile([128, 2, 512], BF16, tag="aT")
                for mo in range(2):
                    nfull = 4 * Q           # full-width key chunks, no mask
                    out_ps = [
                        psout.tile([65, 512], F32, tag="out", name=f"out_ps{_h}")
                        for _h in range(2)
                    ]
                    for jc in range(nfull):
                        sc = psmm.tile([128, 2, 512], F32, tag="mm")
                        for hp in range(2):
                            nc.tensor.matmul(
                                sc[:, hp, :],
                                (kts[mo][jc // 8][hp * 64:(hp + 1) * 64,
                                       (jc % 8) * 128:(jc % 8 + 1) * 128]),
                                (qts[mo][Q // 2][hp * 64:(hp + 1) * 64,
                                       (Q % 2) * 512:(Q % 2 + 1) * 512]),
                                start=True,
                                stop=True,
                                skip_group_check=True,
                            )
                        ex = work.tile([128, 2, 512], BF16, tag="exp")
                        nc.scalar.activation(ex[:], sc[:], EXP, scale=SCALE)
                        for hp in range(2):
                            h = 2 * mo + hp
                            nc.tensor.matmul(
                                out_ps[hp][:],
                                (vts[jc][:, h * 65:(h + 1) * 65]),
                                (ex[:, hp, :]),
                                start=(jc == 0),
                                stop=False,
                                skip_group_check=True,
                            )
                    # diagonal 512x512 region: key chunk kc covers queries
                    # kc*128..512; only the leading 128 block needs the mask
                    for kc in range(4):
                        jc = 4 * Q + kc
                        qw = (4 - kc) * 128   # query width
                        q0 = (Q % 2) * 512 + kc * 128
                        sc = psmm.tile([128, 2, 512], F32, tag="mm")
                        for hp in range(2):
                            nc.tensor.matmul(
                                sc[:, hp, 0:qw],
                                (kts[mo][jc // 8][hp * 64:(hp + 1) * 64,
                                       (jc % 8) * 128:(jc % 8 + 1) * 128]),
                                (qts[mo][Q // 2][hp * 64:(hp + 1) * 64,
                                       q0:q0 + qw]),
                                start=True,
                                stop=True,
                                skip_group_check=True,
                            )
                        ex = work.tile([128, 2, 512], BF16, tag="exp")
                        nc.scalar.activation(
                            ex[:, :, 0:qw], sc[:, :, 0:qw], EXP, scale=SCALE
                        )
                        nc.vector.tensor_mul(
                            ex[:, :, 0:128], ex[:, :, 0:128], maskt[:]
                        )
                        for hp in range(2):
                            h = 2 * mo + hp
                            nc.tensor.matmul(
                                out_ps[hp][:, kc * 128:512],
                                (vts[jc][:, h * 65:(h + 1) * 65]),
                                (ex[:, hp, 0:qw]),
                                start=(jc == 0),
                                stop=(kc == 3),
                                skip_group_check=True,
                            )
                    for hp in range(2):
                        den = rpool.tile([1, 512], F32, tag="den")
                        nc.vector.tensor_copy(den[:], out_ps[hp][64:65, :])
                        rd_f = rpool.tile([1, 512], F32, tag="rdf")
                        nc.vector.reciprocal_approx_fast(out=rd_f[:], in_=den[:])
                        rd = rpool.tile([1, 512], BF16, tag="rd")
                        nc.vector.tensor_copy(rd[:], rd_f[:])
                        # broadcast 1/denom across 64 partitions via K=1 matmul
                        rdb = psmm.tile([64, 512], F32, tag="mm")
                        nc.tensor.matmul(
                            rdb[:], (ones64[:]), (rd[:]),
                            start=True, stop=True, skip_group_check=True,
                        )
                        att = work.tile([64, 512], BF16, tag="att")
                        nc.vector.tensor_copy(att[:], out_ps[hp][0:64, :])
                        rdbs = work.tile([64, 512], BF16, tag="rdbs")
                        nc.vector.tensor_copy(rdbs[:], rdb[:])
                        nc.vector.tensor_mul(
                            aT[hp * 64:(hp + 1) * 64, mo, :],
                            att[:],
                            rdbs[:],
                        )

                # out-proj for this query block: partial[s, :] = a @ woT
                for so in range(4):
                    osb = opool.tile([128, D], F32, tag="osb")
                    po = psmm.tile([128, 2, 512], F32, tag="mm")
                    for co in range(2):
                        for nt in range(2):
                            nc.tensor.matmul(
                                po[:, nt, :],
                                (aT[:, co, so * 128:(so + 1) * 128]),
                                (wo[:, co, nt * 512:(nt + 1) * 512]),
                                start=(co == 0),
                                stop=(co == 1),
                                skip_group_check=True,
                            )
                    nc.vector.tensor_copy(
                        osb[:], po.rearrange("p a n -> p (a n)")
                    )
                    nc.sync.dma_start(
                        out_d.rearrange("(a p) n -> p a n", p=128)[:, Q * 4 + so, :],
                        osb[:],
                    )

    nc.compile()
    return nc


_NC = None


def _get_nc():
    global _NC
    if _NC is None:
        _NC = build_bass()
    return _NC


def _causal_mask():
    j = np.arange(128)[:, None, None]
    i = np.arange(128)[None, None, :]
    return np.broadcast_to(j <= i, (128, 2, 128)).astype(ml_dtypes.bfloat16)


def kernel(in_features, Wq, Wk, Wv, Wo):
    global LAST_RESULTS
    nc = _get_nc()

    bf = ml_dtypes.bfloat16
    x = np.asarray(in_features, np.float32)
    Wq = np.asarray(Wq, np.float32)
    Wk = np.asarray(Wk, np.float32)
    Wv = np.asarray(Wv, np.float32)
    Wo = np.asarray(Wo, np.float32)
    mask = _causal_mask()

    in_maps = []
    for c in range(N_CORES):
        b, g = divmod(c, 4)
        cols = slice(g * GC, (g + 1) * GC)
        in_maps.append({
            "xT": np.ascontiguousarray(x[b].T).astype(bf),
            "wqT": np.ascontiguousarray(Wq.T[:, cols]).astype(bf),
            "wkT": np.ascontiguousarray(Wk.T[:, cols]).astype(bf),
            "wvT": np.ascontiguousarray(Wv.T[:, cols]).astype(bf),
            "woT": np.ascontiguousarray(Wo[:, cols].T).astype(bf),
            "mask": mask,
        })

    res = bass_utils.run_bass_kernel_spmd(
        nc, in_maps, core_ids=list(range(N_CORES)), trace=TRACE,
    )
    LAST_RESULTS = res
    parts = [res.results[c]["out"] for c in range(N_CORES)]
    out = np.stack([
        parts[4 * b] + parts[4 * b + 1] + parts[4 * b + 2] + parts[4 * b + 3]
        for b in range(B)
    ]).astype(np.float32)
    return out
